# revision 27
# baseline (speedup 1.0000x reference)
"""KoLeo-loss kernel for 8 Trainium2 NeuronCores — fp8 DoubleRow version.

Reference computation (x of shape [B=16384, D=256] f32):
    xn   = x / ||x||_row
    gram = xn @ xn.T
    min_dist_i = min_{j != i} sqrt(clip(2 - 2*gram_ij, 0))
    loss = -mean(log(min_dist + 1e-8))

Strategy (one identical SPMD program on 8 cores):
  Host prep (O(B*D), cheap): normalize rows, quantize to fp8 e4m3 with
  scale 64, transpose to the feature-major layout xT8[p, k, j] =
  fp8(64 * xn[j, 128k+p]), and roll per core so core c's 2048 query rows
  are local columns 0..2047.  Also ships two [128,128] fp8 constants
  (+64*I, -64*I) used to cancel the self-match diagonal.

  Device (O(B^2*D), the 99.4% of FLOPs):
    - gram tiles via DoubleRow fp8 matmuls: K=256 contracts in a single
      pass (lhsT [128,2,128] stationary, rhs [128,2,512] moving), psum
      value = 4096 * gram.
    - the one bank per query chunk holding the self-match gets an extra
      accumulated matmul (-64I).T @ (+64I) = -4096*I, zeroing the
      diagonal (safely below the row max ~0.3*4096).
    - drain with zero copies: for each 4-bank span, ACT retires 2 banks
      with Exp(scale*psum + bias) + accumulate (a log-sum-exp whose
      softmax bias < ln(2)/T_LSE), DVE retires 2 banks with pool_max.
  Host finish: gmax = max(pool_max/4096, C_LSE + log(sum S)/T_LSE),
  min_dist = sqrt(2-2*gmax), loss = -mean(log(min_dist + 1e-8)).
"""

import sys

if "/opt/trn_rl_repo" not in sys.path:
    sys.path.insert(0, "/opt/trn_rl_repo")

import numpy as np

D = 256
P = 128
BANK = 512  # psum bank width in f32 elements
B_FULL = 16384
N_CORES = 8
QPC = B_FULL // N_CORES  # queries per core
N_MC = QPC // P  # query chunks per core (16)
N_BANKS = B_FULL // BANK  # gram banks per query chunk (32)
N_SP = N_BANKS // 4  # 4-bank spans per query chunk (8)
CHUNK = 4 * BANK  # column chunk = one span width (2048)

SCALE = 64.0  # fp8 quantization scale; psum = SCALE^2 * gram = 4096*gram
T_LSE = 256.0  # log-sum-exp sharpness (bias < ln2/T per row)
C_LSE = 0.45  # shift; must sit above every row-max gram (~0.42 max)


def build_nc():
    import concourse.mybir as mybir
    import concourse.tile as tile
    from concourse import bacc

    dt = mybir.dt
    AF = mybir.ActivationFunctionType
    DR = mybir.MatmulPerfMode.DoubleRow

    nc = bacc.Bacc(None)
    xT_in = nc.declare_dram_parameter(
        "xT8", [P, 2, B_FULL], dt.float8e4, isOutput=False
    )
    idp_in = nc.declare_dram_parameter("idp", [P, P], dt.float8e4, isOutput=False)
    idn_in = nc.declare_dram_parameter("idn", [P, P], dt.float8e4, isOutput=False)
    bias_in = nc.declare_dram_parameter("biasv", [P, 1], dt.float32, isOutput=False)
    # stats[:, :, 0:N_SP] = per-span pool maxes; [:, :, N_SP:] = exp sums.
    out_st = nc.declare_dram_parameter(
        "stats", [P, N_MC, 2 * N_SP], dt.float32, isOutput=True
    )

    with tile.TileContext(nc) as tc:
        with (
            tc.tile_pool(name="persist", bufs=1) as persist,
            tc.tile_pool(name="scratch", bufs=4) as scr,
            tc.tile_pool(name="ps", bufs=4, space="PSUM") as psp,
        ):
            xT = persist.tile([P, 2, B_FULL], dt.float8e4)
            idp = persist.tile([P, P], dt.float8e4)
            idn = persist.tile([P, P], dt.float8e4)
            stats = persist.tile([P, N_MC, 2 * N_SP], dt.float32)
            biasT = persist.tile([P, 1], dt.float32)
            wtile = persist.tile([P, 2 * P], dt.float8e4)
            nc.scalar.memzero(wtile)

            # Half-chunk DMAs (1024 cols): span (ch, mc)'s A-side only
            # needs the first half of chunk ch, so compute starts sooner.
            HC = CHUNK // 2
            nc.sync.dma_start(out=xT[:, :, 0:HC], in_=xT_in[:, :, 0:HC])
            nc.sync.dma_start(out=idp, in_=idp_in[:, :])
            nc.sync.dma_start(out=idn, in_=idn_in[:, :])
            nc.sync.dma_start(out=biasT, in_=bias_in[:, :])
            for h in range(1, 2 * N_SP):
                nc.sync.dma_start(
                    out=xT[:, :, h * HC : (h + 1) * HC],
                    in_=xT_in[:, :, h * HC : (h + 1) * HC],
                )

            # PE warmup on the zeroed tile (no DMA dependency): keeps the
            # HAM busy-window warm while chunk 0 lands.
            wps = psp.tile([P, BANK], dt.float32, tag="pA", bufs=2, name="warm")
            for _ in range(12):
                nc.tensor.matmul(
                    wps[:, 0:P], wtile[:, 0:P], wtile[:, P : 2 * P],
                    start=True, stop=True,
                )

            act_scale = float(T_LSE / (SCALE * SCALE))

            # Span (ch, mc): banks 4ch..4ch+3 of query chunk mc.
            # Banks 4ch, 4ch+1 -> ptA, drained by ACT exp+accum (LSE).
            # Banks 4ch+2, 4ch+3 -> ptB, drained by DVE pool_max.
            for ch in range(N_SP):
                for mc in range(N_MC):
                    lhsT = xT[:, :, mc * P : (mc + 1) * P]
                    db = mc // 4  # global bank holding this mc's diagonal
                    off = (mc % 4) * P  # its within-bank column offset
                    for half, tag in ((0, "pA"), (1, "pB")):
                        pt = psp.tile([P, 2, BANK], dt.float32, tag=tag, bufs=2)
                        for s in range(2):
                            b = 4 * ch + 2 * half + s
                            hasd = b == db
                            nc.tensor.matmul(
                                pt[:, s, :],
                                lhsT,
                                xT[:, :, b * BANK : (b + 1) * BANK],
                                start=True,
                                stop=not hasd,
                                perf_mode=DR,
                            )
                            if hasd:
                                nc.tensor.matmul(
                                    pt[:, s, off : off + P],
                                    idn,
                                    idp,
                                    start=False,
                                    stop=True,
                                    skip_group_check=True,
                                )
                        if half == 0:
                            # Exp writes back in-place to PSUM: skips the
                            # SBUF trash write; only accum_out is consumed.
                            nc.scalar.activation(
                                out=pt,
                                in_=pt,
                                func=AF.Exp,
                                scale=act_scale,
                                bias=biasT,
                                accum_out=stats[:, mc, N_SP + ch : N_SP + ch + 1],
                            )
                        else:
                            nc.vector.tensor_reduce(
                                stats[:, mc, ch : ch + 1],
                                pt,
                                axis=mybir.AxisListType.XY,
                                op=mybir.AluOpType.max,
                            )

            nc.sync.dma_start(out=out_st[:, :, :], in_=stats)

    nc.compile()
    return nc


_NC_CACHE = {}


def _get_nc():
    if "nc" not in _NC_CACHE:
        _NC_CACHE["nc"] = build_nc()
    return _NC_CACHE["nc"]


LAST_RESULT = None  # BassKernelResults of the most recent run (for profiling)


def kernel(student_output: np.ndarray) -> np.ndarray:
    import os

    import ml_dtypes
    from concourse.bass_utils import run_bass_kernel_spmd

    global LAST_RESULT
    x = np.ascontiguousarray(student_output, dtype=np.float32)
    assert x.shape == (B_FULL, D)

    # Host prep: normalize rows, fp8-quantize, feature-major transpose.
    norm = np.maximum(np.sqrt((x.astype(np.float64) ** 2).sum(axis=1)), 1e-12)
    xn = (x / norm[:, None].astype(np.float32)).astype(np.float32)
    xq = (xn * np.float32(SCALE)).astype(ml_dtypes.float8_e4m3)
    # xT8[p, k, j] = xq[j, 128k + p]
    xT8 = np.ascontiguousarray(xq.reshape(B_FULL, 2, P).transpose(2, 1, 0))
    ident = np.eye(P, dtype=np.float32)
    idp = (ident * SCALE).astype(ml_dtypes.float8_e4m3)
    idn = (-ident * SCALE).astype(ml_dtypes.float8_e4m3)

    nc = _get_nc()
    biasv = np.full((P, 1), -T_LSE * C_LSE, dtype=np.float32)
    in_maps = [
        {
            "xT8": np.roll(xT8, -c * QPC, axis=2),
            "idp": idp,
            "idn": idn,
            "biasv": biasv,
        }
        for c in range(N_CORES)
    ]
    trace = bool(int(os.environ.get("KOLEO_TRACE", "0")))
    res = run_bass_kernel_spmd(
        nc, in_maps, core_ids=list(range(N_CORES)), trace=trace
    )
    LAST_RESULT = res

    s2 = SCALE * SCALE
    gmax = np.empty(B_FULL, dtype=np.float64)
    for c in range(N_CORES):
        st = res.results[c]["stats"]  # [128, N_MC, 16]: 8 maxes then 8 sums
        gm = st[:, :, :N_SP]
        sa = st[:, :, N_SP:]
        m_pool = gm.astype(np.float64).max(axis=2) / s2  # [128, N_MC]
        S = sa.astype(np.float64).sum(axis=2)  # [128, N_MC]
        with np.errstate(divide="ignore"):
            m_lse = C_LSE + np.log(S) / T_LSE
        m = np.maximum(m_pool, m_lse)  # [128(p), N_MC(mc)]
        # query local row = mc*128 + p
        gmax[c * QPC : (c + 1) * QPC] = m.T.ravel()

    min_dist = np.sqrt(np.clip(2.0 - 2.0 * gmax, 0.0, None))
    loss = -np.mean(np.log(min_dist + 1e-8))
    return np.float32(loss)


if __name__ == "__main__":
    rng = np.random.default_rng(0)
    x = rng.standard_normal((B_FULL, D), dtype=np.float32)
    out = kernel(x)
    print("loss:", out)


# revision 28
# speedup vs baseline: 1.0264x; 1.0264x over previous
"""KoLeo-loss kernel for 8 Trainium2 NeuronCores — fp8 DoubleRow version.

Reference computation (x of shape [B=16384, D=256] f32):
    xn   = x / ||x||_row
    gram = xn @ xn.T
    min_dist_i = min_{j != i} sqrt(clip(2 - 2*gram_ij, 0))
    loss = -mean(log(min_dist + 1e-8))

Strategy (one identical SPMD program on 8 cores):
  Host prep (O(B*D), cheap): normalize rows, quantize to fp8 e4m3 with
  scale 64, transpose to the feature-major layout xT8[p, k, j] =
  fp8(64 * xn[j, 128k+p]), and roll per core so core c's 2048 query rows
  are local columns 0..2047.  Also ships two [128,128] fp8 constants
  (+64*I, -64*I) used to cancel the self-match diagonal.

  Device (O(B^2*D), the 99.4% of FLOPs):
    - gram tiles via DoubleRow fp8 matmuls: K=256 contracts in a single
      pass (lhsT [128,2,128] stationary, rhs [128,2,512] moving), psum
      value = 4096 * gram.
    - the one bank per query chunk holding the self-match gets an extra
      accumulated matmul (-64I).T @ (+64I) = -4096*I, zeroing the
      diagonal (safely below the row max ~0.3*4096).
    - drain with zero copies: for each 4-bank span, ACT retires 2 banks
      with Exp(scale*psum + bias) + accumulate (a log-sum-exp whose
      softmax bias < ln(2)/T_LSE), DVE retires 2 banks with pool_max.
  Host finish: gmax = max(pool_max/4096, C_LSE + log(sum S)/T_LSE),
  min_dist = sqrt(2-2*gmax), loss = -mean(log(min_dist + 1e-8)).
"""

import sys

if "/opt/trn_rl_repo" not in sys.path:
    sys.path.insert(0, "/opt/trn_rl_repo")

import numpy as np

D = 256
P = 128
BANK = 512  # psum bank width in f32 elements
B_FULL = 16384
N_CORES = 8
QPC = B_FULL // N_CORES  # queries per core
N_MC = QPC // P  # query chunks per core (16)
N_BANKS = B_FULL // BANK  # gram banks per query chunk (32)
N_SP = N_BANKS // 4  # 4-bank spans per query chunk (8)
CHUNK = 4 * BANK  # column chunk = one span width (2048)

SCALE = 64.0  # fp8 quantization scale; psum = SCALE^2 * gram = 4096*gram
T_LSE = 256.0  # log-sum-exp sharpness (bias < ln2/T per row)
C_LSE = 0.45  # shift; must sit above every row-max gram (~0.42 max)


def build_nc():
    import concourse.mybir as mybir
    import concourse.tile as tile
    from concourse import bacc

    dt = mybir.dt
    AF = mybir.ActivationFunctionType
    DR = mybir.MatmulPerfMode.DoubleRow

    nc = bacc.Bacc(None)
    xT_in = nc.declare_dram_parameter(
        "xT8", [P, 2, B_FULL], dt.float8e4, isOutput=False
    )
    idp_in = nc.declare_dram_parameter("idp", [P, P], dt.float8e4, isOutput=False)
    idn_in = nc.declare_dram_parameter("idn", [P, P], dt.float8e4, isOutput=False)
    bias_in = nc.declare_dram_parameter("biasv", [P, 1], dt.float32, isOutput=False)
    # stats[:, :, 0:N_SP] = per-span pool maxes; [:, :, N_SP:] = exp sums.
    out_st = nc.declare_dram_parameter(
        "stats", [P, N_MC, 2 * N_SP], dt.float32, isOutput=True
    )

    with tile.TileContext(nc) as tc:
        with (
            tc.tile_pool(name="persist", bufs=1) as persist,
            tc.tile_pool(name="scratch", bufs=4) as scr,
            tc.tile_pool(name="ps", bufs=4, space="PSUM") as psp,
        ):
            xT = persist.tile([P, 2, B_FULL], dt.float8e4)
            idp = persist.tile([P, P], dt.float8e4)
            idn = persist.tile([P, P], dt.float8e4)
            stats = persist.tile([P, N_MC, 2 * N_SP], dt.float32)
            biasT = persist.tile([P, 1], dt.float32)
            wtile = persist.tile([P, 2 * P], dt.float8e4)
            nc.scalar.memzero(wtile)

            # Half-chunk DMAs (1024 cols): span (ch, mc)'s A-side only
            # needs the first half of chunk ch, so compute starts sooner.
            HC = CHUNK // 2
            nc.sync.dma_start(out=xT[:, :, 0:HC], in_=xT_in[:, :, 0:HC])
            nc.sync.dma_start(out=idp, in_=idp_in[:, :])
            nc.sync.dma_start(out=idn, in_=idn_in[:, :])
            nc.sync.dma_start(out=biasT, in_=bias_in[:, :])
            for h in range(1, 2 * N_SP):
                nc.sync.dma_start(
                    out=xT[:, :, h * HC : (h + 1) * HC],
                    in_=xT_in[:, :, h * HC : (h + 1) * HC],
                )

            # PE warmup on the zeroed tile (no DMA dependency): keeps the
            # HAM busy-window warm while chunk 0 lands.
            wps = psp.tile([P, BANK], dt.float32, tag="pA", bufs=2, name="warm")
            for _ in range(12):
                nc.tensor.matmul(
                    wps[:, 0:P], wtile[:, 0:P], wtile[:, P : 2 * P],
                    start=True, stop=True,
                )

            act_scale = float(T_LSE / (SCALE * SCALE))

            # Span (ch, mc): banks 4ch..4ch+3 of query chunk mc.
            # Banks 4ch, 4ch+1 -> ptA, drained by ACT exp+accum (LSE).
            # Banks 4ch+2, 4ch+3 -> ptB, drained by DVE pool_max.
            for ch in range(N_SP):
                for mc in range(N_MC):
                    lhsT = xT[:, :, mc * P : (mc + 1) * P]
                    db = mc // 4  # global bank holding this mc's diagonal
                    off = (mc % 4) * P  # its within-bank column offset
                    for half, tag in ((0, "pA"), (1, "pB")):
                        pt = psp.tile([P, 2, BANK], dt.float32, tag=tag, bufs=2)
                        for s in range(2):
                            b = 4 * ch + 2 * half + s
                            hasd = b == db
                            nc.tensor.matmul(
                                pt[:, s, :],
                                lhsT,
                                xT[:, :, b * BANK : (b + 1) * BANK],
                                start=True,
                                stop=not hasd,
                                perf_mode=DR,
                            )
                            if hasd:
                                nc.tensor.matmul(
                                    pt[:, s, off : off + P],
                                    idn,
                                    idp,
                                    start=False,
                                    stop=True,
                                    skip_group_check=True,
                                )
                        if half == 0:
                            trash = scr.tile(
                                [P, 2, BANK], dt.float16, tag="trash", bufs=4
                            )
                            nc.scalar.activation(
                                out=trash,
                                in_=pt,
                                func=AF.Exp,
                                scale=act_scale,
                                bias=biasT,
                                accum_out=stats[:, mc, N_SP + ch : N_SP + ch + 1],
                            )
                        else:
                            nc.vector.tensor_reduce(
                                stats[:, mc, ch : ch + 1],
                                pt,
                                axis=mybir.AxisListType.XY,
                                op=mybir.AluOpType.max,
                            )

            nc.sync.dma_start(out=out_st[:, :, :], in_=stats)

    nc.compile()
    return nc


_NC_CACHE = {}


def _get_nc():
    if "nc" not in _NC_CACHE:
        _NC_CACHE["nc"] = build_nc()
    return _NC_CACHE["nc"]


LAST_RESULT = None  # BassKernelResults of the most recent run (for profiling)


def kernel(student_output: np.ndarray) -> np.ndarray:
    import os

    import ml_dtypes
    from concourse.bass_utils import run_bass_kernel_spmd

    global LAST_RESULT
    x = np.ascontiguousarray(student_output, dtype=np.float32)
    assert x.shape == (B_FULL, D)

    # Host prep: normalize rows, fp8-quantize, feature-major transpose.
    norm = np.maximum(np.sqrt((x.astype(np.float64) ** 2).sum(axis=1)), 1e-12)
    xn = (x / norm[:, None].astype(np.float32)).astype(np.float32)
    xq = (xn * np.float32(SCALE)).astype(ml_dtypes.float8_e4m3)
    # xT8[p, k, j] = xq[j, 128k + p]
    xT8 = np.ascontiguousarray(xq.reshape(B_FULL, 2, P).transpose(2, 1, 0))
    ident = np.eye(P, dtype=np.float32)
    idp = (ident * SCALE).astype(ml_dtypes.float8_e4m3)
    idn = (-ident * SCALE).astype(ml_dtypes.float8_e4m3)

    nc = _get_nc()
    biasv = np.full((P, 1), -T_LSE * C_LSE, dtype=np.float32)
    in_maps = [
        {
            "xT8": np.roll(xT8, -c * QPC, axis=2),
            "idp": idp,
            "idn": idn,
            "biasv": biasv,
        }
        for c in range(N_CORES)
    ]
    trace = bool(int(os.environ.get("KOLEO_TRACE", "0")))
    res = run_bass_kernel_spmd(
        nc, in_maps, core_ids=list(range(N_CORES)), trace=trace
    )
    LAST_RESULT = res

    s2 = SCALE * SCALE
    gmax = np.empty(B_FULL, dtype=np.float64)
    for c in range(N_CORES):
        st = res.results[c]["stats"]  # [128, N_MC, 16]: 8 maxes then 8 sums
        gm = st[:, :, :N_SP]
        sa = st[:, :, N_SP:]
        m_pool = gm.astype(np.float64).max(axis=2) / s2  # [128, N_MC]
        S = sa.astype(np.float64).sum(axis=2)  # [128, N_MC]
        with np.errstate(divide="ignore"):
            m_lse = C_LSE + np.log(S) / T_LSE
        m = np.maximum(m_pool, m_lse)  # [128(p), N_MC(mc)]
        # query local row = mc*128 + p
        gmax[c * QPC : (c + 1) * QPC] = m.T.ravel()

    min_dist = np.sqrt(np.clip(2.0 - 2.0 * gmax, 0.0, None))
    loss = -np.mean(np.log(min_dist + 1e-8))
    return np.float32(loss)


if __name__ == "__main__":
    rng = np.random.default_rng(0)
    x = rng.standard_normal((B_FULL, D), dtype=np.float32)
    out = kernel(x)
    print("loss:", out)
